# revision 30
# baseline (speedup 1.0000x reference)
"""Trainium2 Bass kernel for nn_Net_71270687310327 (scatter_memory).

Computation (see reference):
  - keys = (timings+1)*512 + slot_index, with argmin(surprise*0.9) slot's key
    overridden to its slot index (forces rank 0, stable-sort tiebreak exact).
  - rank[b,m] = #{m' : key[b,m'] < key[b,m]}  (all keys distinct)
  - pred_in = [sorted memory rows | timing bits], fed to a 4-layer MLP.

Optimized v2 design (vs the fp32 replicated baseline):
  - All matmul operands fp16 (host-side cast): 4x PE matmul rate, 2x less
    W0 DMA traffic. PSUM accumulation stays fp32; keys/rank math stays fp32.
  - W0 shard streamed into a large SBUF staging pool from t=0 on the sync
    HW DGE queue, overlapping the whole ranking phase.
  - Ranking batch-sharded: each core ranks only its 4 batches, then ONE
    AllToAll redistributes (sorted key, slot) from batch-sharded to
    rank-sharded layout - exactly what each core needs (its 64 ranks x all
    32 batches).
  - Gather uses dma_gather(transpose=True) on fp16 rows: fuses the memory
    row gather with the pred_in^T transpose (no PE transposes, no repack),
    in 8 chunks that pipeline with the main matmul.
  - W1/W2/Wout preloaded fp16-resident at t=0; dense tail runs right after
    the partial-h AllReduce.

The same program runs on all 8 cores (SPMD); all per-core differences are
carried by per-core input constants (W0 shard, selection/offset constants).
"""

import sys

sys.path.insert(0, "/opt/trn_rl_repo")

import numpy as np

import concourse.bass as bass
import concourse.bacc as bacc
import concourse.mybir as mybir
from concourse import tile
from concourse import bass_utils

F32 = mybir.dt.float32
F16 = mybir.dt.float16
I16 = mybir.dt.int16
U8 = mybir.dt.uint8
ALU = mybir.AluOpType
ACTF = mybir.ActivationFunctionType

B, M, V, H, TD = 32, 512, 256, 1024, 10
NC = 8
RPC = M // NC              # 64 ranks per core
BPC = B // NC              # 4 batches ranked per core
MEMROWS = B * M            # 16384
MEMP = MEMROWS + B         # 16416 gather-source rows (memory rows + x rows)
W0S_ROWS = RPC * V + RPC * TD   # 17024 = 133*128
NKT = W0S_ROWS // 128      # 133 k-tiles (128 mem + 5 bits)
NBT = RPC * TD // 128      # 5 bits k-tiles
NST = 44                   # W0 staging tiles in SBUF (44*256KB = 11.5MB)
NIDX = RPC * B             # 2048 gather items
GCH = 8                    # gather chunks
IPG = NIDX // GCH          # 256 idxs per chunk


def build_program(stage="full"):
    lvl = {"rank": 0, "skb": 1, "gt": 2, "full": 3}[stage]
    nc = bacc.Bacc(
        "TRN2",
        target_bir_lowering=False,
        debug=False,
        enable_asserts=False,
        num_devices=NC,
    )

    def din(name, shape, dtype=F32):
        return nc.dram_tensor(name, list(shape), dtype, kind="ExternalInput").ap()

    memp = din("memp", (MEMP, V), F16)
    timings = din("timings", (B, M))
    msur = din("msur", (B, M))
    w0s = din("W0s", (W0S_ROWS, H), F16)
    w1 = din("W1h", (H, H), F16)
    w2 = din("W2h", (H, H), F16)
    wout = din("Wouth", (H, V), F16)
    b0r = din("b0r", (B, H))
    b1r = din("b1r", (B, H))
    b2r = din("b2r", (B, H))
    boutr = din("boutr", (B, V))
    c_iota = din("c_iota", (B, M))
    c_rrow = din("c_rrow512", (128, M))
    c_eselc = din("c_eselc", (B, 4 * 128))
    c_esel4x = din("c_esel4x", (B, 4))
    c_eye = din("c_eye", (B, B))
    c_slotnum = din("c_slotnum", (128, 4))
    c_selsk = din("c_selsk", (2 * B, B), F16)
    c_selj = din("c_selj64", (2 * B, 2 * 128))
    c_skbsel = din("c_skbsel", (128, 4 * RPC))
    c_amask = din("c_amask", (128, 128))
    c_coff = din("c_coff", (128, 128))

    out = nc.dram_tensor("out", [B, V], F32, kind="ExternalOutput").ap()
    dbg = (nc.dram_tensor("dbg", [128, 512], F32, kind="ExternalOutput").ap()
           if stage != "full" else None)

    with tile.TileContext(nc) as tc:
        with (
            tc.tile_pool(name="const", bufs=1) as constp,
            tc.tile_pool(name="state", bufs=1) as state,
            tc.tile_pool(name="w0t", bufs=NST) as w0p,
            tc.tile_pool(name="gt", bufs=GCH) as gtp,
            tc.tile_pool(name="krep", bufs=2) as krepp,
            tc.tile_pool(name="pt", bufs=2) as ptp,
            tc.tile_pool(name="kslot", bufs=2) as kslotp,
            tc.tile_pool(name="pkrep", bufs=2, space="PSUM") as pkrepp,
            tc.tile_pool(name="psort", bufs=2, space="PSUM") as psortp,
            tc.tile_pool(name="pmisc", bufs=2, space="PSUM") as pmiscp,
            tc.tile_pool(name="ph", bufs=1, space="PSUM") as php,
            tc.tile_pool(name="dram", bufs=1, space="DRAM") as dramp,
        ):
            # ---- W0 stream: issue ALL tile DMAs up-front on the sync HW
            # queue. First NST run at t=0 into the staging pool; the rest
            # stall on slot recycling and stream as the main loop consumes.
            w0_tiles = []
            for kt in range(NKT):
                t = w0p.tile([128, H], F16, tag="w0t")
                nc.sync.dma_start(t[:], w0s[kt * 128:(kt + 1) * 128, :])
                w0_tiles.append(t)

            # ---- constants / state / tail weights on the scalar HW queue
            def load(pool, ap, dtype=None):
                t = pool.tile(list(ap.shape), dtype or ap.dtype,
                              tag=f"ld_{ap.tensor.name}")
                nc.scalar.dma_start(t[:], ap)
                return t

            iota = load(constp, c_iota)
            t_sb = load(state, timings)
            ms_sb = load(state, msur)
            eselc = load(constp, c_eselc)
            esel4x = load(constp, c_esel4x)
            rrow = load(constp, c_rrow)
            slotnum = load(constp, c_slotnum)
            selsk = load(constp, c_selsk)
            selj = load(constp, c_selj)
            skbsel = load(constp, c_skbsel)
            amask = load(constp, c_amask)
            coff = load(constp, c_coff)
            eye = load(constp, c_eye)
            b0s = load(constp, b0r)
            b1s = load(constp, b1r)
            b2s = load(constp, b2r)
            bouts = load(constp, boutr)

            # resident fp16 tail weights: W1/W2 as 8 k-tiles, Wout as 8
            w1s = constp.tile([128, 8 * H], F16, tag="w1s")
            nc.scalar.dma_start(
                w1s.rearrange("p (k f) -> p k f", k=8),
                w1.rearrange("(k p) f -> p k f", k=8))
            w2s = constp.tile([128, 8 * H], F16, tag="w2s")
            nc.scalar.dma_start(
                w2s.rearrange("p (k f) -> p k f", k=8),
                w2.rearrange("(k p) f -> p k f", k=8))
            wos = constp.tile([128, 8 * V], F16, tag="wos")
            nc.scalar.dma_start(
                wos.rearrange("p (k f) -> p k f", k=8),
                wout.rearrange("(k p) f -> p k f", k=8))

            # ---- stage A: keys (all 32 batches; cheap) ------------------
            msur2 = state.tile([B, M], F32, tag="msur2")
            nc.vector.tensor_scalar(msur2[:], ms_sb[:], 0.9, None, ALU.mult)
            minv = state.tile([B, 1], F32, tag="minv")
            nc.vector.tensor_reduce(minv[:], msur2[:], axis=mybir.AxisListType.X,
                                    op=ALU.min)
            mask = state.tile([B, M], U8, tag="mask")
            nc.vector.tensor_scalar(mask[:], msur2[:], minv[:], None, ALU.is_equal)
            cand = state.tile([B, M], F32, tag="cand")
            nc.vector.memset(cand[:], 1.0e9)
            nc.vector.copy_predicated(cand[:], mask[:], iota[:])
            idx0 = state.tile([B, 1], F32, tag="idx0")
            nc.vector.tensor_reduce(idx0[:], cand[:], axis=mybir.AxisListType.X,
                                    op=ALU.min)

            keys = state.tile([B, M], F32, tag="keys")
            nc.vector.tensor_scalar(keys[:], t_sb[:], 512.0, 512.0, ALU.mult,
                                    ALU.add)
            nc.vector.tensor_tensor(keys[:], keys[:], iota[:], ALU.add)
            mask2 = state.tile([B, M], U8, tag="mask2")
            nc.vector.tensor_scalar(mask2[:], iota[:], idx0[:], None, ALU.is_equal)
            nc.vector.copy_predicated(keys[:], mask2[:], iota[:])

            # ---- stage B: rank my 4 batches; extract (sorted key, slot) --
            rank_sb = state.tile([128, 4 * BPC], F32, tag="rank")
            scratch = state.tile([128, M], F32, tag="scratch")
            psort_sb = state.tile([2, BPC * M], F32, tag="psort_sb")
            for bi in range(BPC):
                pk = pkrepp.tile([128, M], F32, tag="pkrep")
                nc.tensor.matmul(pk[:], eselc[:, bi * 128:(bi + 1) * 128],
                                 keys[:], start=True, stop=True)
                krep = krepp.tile([128, M], F32, tag="krep")
                nc.vector.tensor_copy(krep[:], pk[:])
                # keysT for my batch: kslot even cols = key^T, odd = slot ids
                pkt = pmiscp.tile([128, 4], F32, tag="pm")
                for mt in range(4):
                    nc.tensor.matmul(pkt[:, mt:mt + 1],
                                     keys[:, mt * 128:(mt + 1) * 128],
                                     esel4x[:, bi:bi + 1], start=True, stop=True)
                kslot = kslotp.tile([128, 8], F32, tag="kslot")
                ksv = kslot.rearrange("p (c two) -> p c two", two=2)
                nc.vector.tensor_copy(ksv[:, :, 0], pkt[:])
                nc.vector.tensor_copy(ksv[:, :, 1], slotnum[:])
                psort_ps = psortp.tile([2, M], F32, tag="psort")
                for mt in range(4):
                    col = bi * 4 + mt
                    nc.vector.tensor_scalar(
                        scratch[:], krep[:], kslot[:, 2 * mt:2 * mt + 1], None,
                        ALU.is_lt, ALU.add,
                        accum_out=rank_sb[:, col:col + 1])
                    pt = ptp.tile([128, M], F32, tag="pt")
                    nc.vector.tensor_scalar(pt[:], rrow[:],
                                            rank_sb[:, col:col + 1], None,
                                            ALU.is_equal)
                    nc.tensor.matmul(psort_ps[:],
                                     kslot[:, 2 * mt:2 * mt + 2], pt[:],
                                     start=(mt == 0), stop=(mt == 3))
                nc.scalar.activation(
                    psort_sb.rearrange("two (d bi r) -> two d bi r",
                                       d=NC, bi=BPC)[:, :, bi, :],
                    psort_ps.rearrange("two (d r) -> two d r", d=NC),
                    ACTF.Copy)

            if stage == "rank":
                for q in range(4):
                    nc.sync.dma_start(dbg[2 * q:2 * q + 2, 0:512],
                                      psort_sb[:, 512 * q:512 * (q + 1)])
                nc.sync.dma_start(dbg[8:8 + 128 - 120, 0:16], rank_sb[0:8, :])

            # ---- stage C: AllGather (sk, slot), then per-core row select --
            # cc_in flat rows (within a src block): two*32 + d*4 + bi
            cc_in = dramp.tile([2, NC, BPC * RPC], F32, tag="cc_in")
            cc_out = dramp.tile([4, 128, RPC], F32, tag="cc_out")
            nc.scalar.dma_start(
                cc_in[:],
                psort_sb.rearrange("two (d br) -> two d br", d=NC))
            nc.gpsimd.collective_compute(
                "AllGather", ALU.bypass,
                replica_groups=[list(range(NC))],
                ins=[cc_in.opt()],
                outs=[cc_out.opt()],
            )
            # skall[p, g*64+r] = gathered row (g*128+p) = (src,two,d,bi) data
            skall = state.tile([128, 4 * RPC], F32, tag="skall")
            nc.scalar.dma_start(skall.rearrange("p (g r) -> p g r", g=4),
                                cc_out.rearrange("g p r -> p g r"))
            # skb rows: (src, two, bi) -> 8*src + 4*two + bi; cols: my 64 ranks
            pskb = pmiscp.tile([2 * B, RPC], F32, tag="pm")
            for g in range(4):
                nc.tensor.matmul(pskb[:], skbsel[:, g * RPC:(g + 1) * RPC],
                                 skall[:, g * RPC:(g + 1) * RPC],
                                 start=(g == 0), stop=(g == 3))
            skb = state.tile([2 * B, RPC], F32, tag="skb")
            nc.vector.tensor_copy(skb[:], pskb[:])

            # ---- stage D: gather idx + timing bits -----------------------
            # idx[p, 2r+j] = 512*b + slot(b, my rank r), b = p%16 + 16j
            idx_sb = state.tile([128, 128], I16, tag="idx")
            idxv = idx_sb.rearrange("p (r j) -> p r j", j=2)
            itmp = state.tile([128, RPC], F32, tag="itmp")
            for j in range(2):
                pidx = pmiscp.tile([128, RPC], F32, tag="pm")
                nc.tensor.matmul(pidx[:], selj[:, j * 128:(j + 1) * 128],
                                 skb[:], start=True, stop=True)
                nc.vector.tensor_tensor(itmp[:], pidx[:],
                                        amask[:, j * RPC:(j + 1) * RPC], ALU.mult)
                nc.vector.tensor_tensor(idxv[:, :, j], itmp[:],
                                        coff[:, j * RPC:(j + 1) * RPC], ALU.add)

            # Timing-bit extraction: MSB-first binary cascade on skb rows
            # (slot rows produce harmless garbage, never selected).
            # u_all[row, rl*10+d] = bit (9+d) of sk(batch(row), rank 64c+rl)
            u_all = state.tile([2 * B, RPC * TD], F16, tag="u_all")
            u_v = u_all.rearrange("row (rl d) -> row rl d", d=TD)
            rem = state.tile([2 * B, RPC], F32, tag="rem")
            tmpu = state.tile([2 * B, RPC], F32, tag="tmpu")
            nc.vector.tensor_copy(rem[:], skb[:])
            for j in range(18, 8, -1):
                ud = u_v[:, :, j - 9]
                nc.vector.tensor_scalar(ud, rem[:], float(2 ** j), None, ALU.is_ge)
                nc.vector.tensor_scalar(tmpu[:], ud, float(2 ** j), None, ALU.mult)
                nc.vector.tensor_tensor(rem[:], rem[:], tmpu[:], ALU.subtract)

            # bits_sb[p, tb*32+b] = u_all[skrow(b), tb*128+p]
            bits_sb = state.tile([128, NBT * B], F16, tag="bits")
            for tb in range(NBT):
                psr = pmiscp.tile([128, B], F32, tag="pm")
                nc.tensor.matmul(psr[:], u_all[:, tb * 128:(tb + 1) * 128],
                                 selsk[:], start=True, stop=True)
                nc.vector.tensor_copy(bits_sb[:, tb * B:(tb + 1) * B], psr[:])

            if stage == "skb":
                nc.sync.dma_start(dbg[0:64, 0:64], skb[:])
                skf = state.tile([128, 128], F32, tag="skf")
                nc.vector.tensor_copy(skf[:], idx_sb[:])
                nc.sync.dma_start(dbg[0:128, 64:192], skf[:])
                bitf = state.tile([128, NBT * B], F32, tag="bitf")
                nc.vector.tensor_copy(bitf[:], bits_sb[:])
                nc.sync.dma_start(dbg[0:128, 192:352], bitf[:])

            # ---- stage E: fused transpose-gather of memory rows ----------
            gts = []
            for k in range(GCH):
                gt = gtp.tile([128, 2, IPG], F16, tag="gt")
                nc.gpsimd.dma_gather(
                    out_ap=gt[:],
                    in_ap=memp,
                    idxs_ap=idx_sb[:, 16 * k:16 * (k + 1)],
                    num_idxs=IPG,
                    num_idxs_reg=IPG,
                    elem_size=V,
                    transpose=True,
                    single_packet=False,
                )
                gts.append(gt)

            if stage == "gt":
                gtf = state.tile([128, 512], F32, tag="gtf")
                nc.vector.tensor_copy(gtf[:], gts[0].rearrange("p c i -> p (c i)"))
                nc.sync.dma_start(dbg[:, 0:512], gtf[:])

            # ---- stage F: main matmul  partial_h = pred_in_shard @ W0s ---
            ph_t = php.tile([B, H], F32, tag="ph")
            for kt in range(NKT):
                if kt < 2 * RPC:
                    r, h2 = kt // 2, kt % 2
                    gt = gts[r // 8]
                    rr = r % 8
                    lhsT = gt[:, h2, rr * B:(rr + 1) * B]
                else:
                    tb = kt - 2 * RPC
                    lhsT = bits_sb[:, tb * B:(tb + 1) * B]
                last = kt == NKT - 1
                w0t = w0_tiles[kt]
                nc.tensor.matmul(ph_t[:, 0:512], lhsT, w0t[:, 0:512],
                                 start=(kt == 0), stop=last)
                nc.tensor.matmul(ph_t[:, 512:1024], lhsT, w0t[:, 512:1024],
                                 start=(kt == 0), stop=last)

            # ---- stage G: AllReduce partial h over the 8 cores -----------
            part_h = state.tile([B, H], F32, tag="part_h")
            nc.vector.tensor_copy(part_h[:], ph_t[:])
            ar_in = dramp.tile([B, H], F32, tag="ar_in")
            ar_out = dramp.tile([B, H], F32, tag="ar_out")
            nc.scalar.dma_start(ar_in[:], part_h[:])
            nc.gpsimd.collective_compute(
                "AllReduce", ALU.add,
                replica_groups=[list(range(NC))],
                ins=[ar_in.opt()],
                outs=[ar_out.opt()],
            )
            h_sb = state.tile([B, H], F32, tag="h_sb")
            nc.scalar.dma_start(h_sb[:], ar_out[:])
            nc.vector.tensor_tensor(h_sb[:], h_sb[:], b0s[:], ALU.add)
            nc.vector.tensor_scalar(h_sb[:], h_sb[:], 0.0, None, ALU.max)

            # ---- stage H: dense tail (replicated on every core) ----------
            def dense(h_in, w_sb, bias_sb, n_out, relu, tag):
                hT = state.tile([128, 8 * B], F16, tag=f"hT_{tag}")
                for kt in range(8):
                    ptt = pmiscp.tile([128, B], F32, tag="pm")
                    nc.tensor.transpose(ptt[:], h_in[:, kt * 128:(kt + 1) * 128],
                                        eye[:])
                    nc.scalar.activation(hT[:, kt * B:(kt + 1) * B], ptt[:],
                                         ACTF.Copy)
                pho = php.tile([B, n_out], F32, tag="ph")
                for kt in range(8):
                    for j0 in range(0, n_out, 512):
                        jn = min(512, n_out - j0)
                        nc.tensor.matmul(
                            pho[:, j0:j0 + jn], hT[:, kt * B:(kt + 1) * B],
                            w_sb[:, kt * n_out + j0:kt * n_out + j0 + jn],
                            start=(kt == 0), stop=(kt == 7))
                h_next = state.tile([B, n_out], F32, tag=f"h_{tag}")
                nc.vector.tensor_tensor(h_next[:], pho[:], bias_sb[:], ALU.add)
                if relu:
                    nc.vector.tensor_scalar(h_next[:], h_next[:], 0.0, None,
                                            ALU.max)
                return h_next

            h1 = dense(h_sb, w1s, b1s, H, True, "l1")
            h2 = dense(h1, w2s, b2s, H, True, "l2")
            logits = dense(h2, wos, bouts, V, False, "lo")
            nc.scalar.dma_start(out, logits[:])

    nc.compile()
    return nc


def make_in_maps(inputs):
    x = np.asarray(inputs["x"], np.float32)
    memory = np.asarray(inputs["memory"], np.float32)
    timings = np.asarray(inputs["memory_timings"], np.float32)
    msur = np.asarray(inputs["memory_surprise"], np.float32)
    W0 = np.asarray(inputs["W0"], np.float32)
    W1 = np.asarray(inputs["W1"], np.float32)
    W2 = np.asarray(inputs["W2"], np.float32)
    Wout = np.asarray(inputs["Wout"], np.float32)
    b0 = np.asarray(inputs["b0"], np.float32)
    b1 = np.asarray(inputs["b1"], np.float32)
    b2 = np.asarray(inputs["b2"], np.float32)
    bout = np.asarray(inputs["bout"], np.float32)

    memp = np.concatenate([memory.reshape(MEMROWS, V), x], 0).astype(np.float16)

    p = np.arange(128)

    # shared constants
    iota = np.broadcast_to(np.arange(M, dtype=np.float32), (B, M)).copy()
    rrow = np.broadcast_to(np.arange(M, dtype=np.float32), (128, M)).copy()
    eye = np.eye(B, dtype=np.float32)
    slotnum = np.empty((128, 4), np.float32)
    for mt in range(4):
        slotnum[:, mt] = p + mt * 128
    # skb rows: (src, two, bi) -> 8*src + 4*two + bi
    # selsk[row, b] = [row == sk-row of batch b]  (sk row = 8*(b//4)+(b%4))
    selsk = np.zeros((2 * B, B), np.float16)
    for b in range(B):
        selsk[8 * (b // 4) + (b % 4), b] = 1.0
    # selj[row, j*128+p] = [row == slot-row of batch p%16+16j]
    selj = np.zeros((2 * B, 2 * 128), np.float32)
    for j in range(2):
        for pp in range(128):
            b = pp % 16 + 16 * j
            selj[8 * (b // 4) + 4 + (b % 4), j * 128 + pp] = 1.0
    shared = {
        "memp": memp,
        "timings": timings,
        "msur": msur,
        "W1h": W1.astype(np.float16),
        "W2h": W2.astype(np.float16),
        "Wouth": Wout.astype(np.float16),
        "b0r": np.broadcast_to(b0, (B, H)).copy(),
        "b1r": np.broadcast_to(b1, (B, H)).copy(),
        "b2r": np.broadcast_to(b2, (B, H)).copy(),
        "boutr": np.broadcast_to(bout, (B, V)).copy(),
        "c_iota": iota, "c_rrow512": rrow, "c_eye": eye,
        "c_slotnum": slotnum, "c_selsk": selsk, "c_selj64": selj,
    }

    in_maps = []
    for core in range(NC):
        w0shard = np.concatenate(
            [W0[core * RPC * V:(core + 1) * RPC * V],
             W0[M * V + core * RPC * TD: M * V + (core + 1) * RPC * TD]],
            0).astype(np.float16)
        eselc = np.zeros((B, 4 * 128), np.float32)
        esel4x = np.zeros((B, 4), np.float32)
        for bi in range(BPC):
            eselc[4 * core + bi, bi * 128:(bi + 1) * 128] = 1.0
            esel4x[4 * core + bi, bi] = 1.0
        # skbsel[p, g*64+orow] = [g*128+p == src*64 + two*32 + core*4 + bi]
        # where orow = 8*src + 4*two + bi
        skbsel = np.zeros((128, 4 * RPC), np.float32)
        for orow in range(64):
            src, two, bi = orow // 8, (orow % 8) // 4, orow % 4
            row = src * 64 + two * 32 + core * 4 + bi
            skbsel[row % 128, (row // 128) * 64 + orow] = 1.0
        amask = np.ones((128, 128), np.float32)
        coffm = np.empty((128, 128), np.float32)
        for j in range(2):
            bcol = (p % 16 + 16 * j).astype(np.float32)
            coffm[:, j * RPC:(j + 1) * RPC] = (512.0 * bcol)[:, None]
            if core == 0:
                amask[:, j * RPC] = 0.0
                coffm[:, j * RPC] = MEMROWS + bcol
        m = dict(shared)
        m["W0s"] = np.ascontiguousarray(w0shard)
        m["c_eselc"] = eselc
        m["c_esel4x"] = esel4x
        m["c_skbsel"] = skbsel
        m["c_amask"] = amask
        m["c_coff"] = coffm
        in_maps.append(m)
    return in_maps


_NC_CACHE = None


def kernel(**inputs) -> np.ndarray:
    global _NC_CACHE
    if _NC_CACHE is None:
        _NC_CACHE = build_program()
    nc = _NC_CACHE
    in_maps = make_in_maps(inputs)
    res = bass_utils.run_bass_kernel_spmd(nc, in_maps, core_ids=list(range(NC)))
    return np.asarray(res.results[0]["out"], np.float32)


if __name__ == "__main__":
    np.random.seed(0)
    build_program(sys.argv[1] if len(sys.argv) > 1 else "full")
    print("build OK")


# revision 40
# speedup vs baseline: 1.0017x; 1.0017x over previous
"""Trainium2 Bass kernel for nn_Net_71270687310327 (scatter_memory).

Computation (see reference):
  - keys = (timings+1)*512 + slot_index, with argmin(surprise*0.9) slot's key
    overridden to its slot index (forces rank 0, stable-sort tiebreak exact).
  - rank[b,m] = #{m' : key[b,m'] < key[b,m]}  (all keys distinct)
  - pred_in = [sorted memory rows | timing bits], fed to a 4-layer MLP.

Optimized v2 design (vs the fp32 replicated baseline):
  - All matmul operands fp16 (host-side cast): 4x PE matmul rate, 2x less
    W0 DMA traffic. PSUM accumulation stays fp32; keys/rank math stays fp32.
  - W0 shard streamed into a large SBUF staging pool from t=0 on the sync
    HW DGE queue, overlapping the whole ranking phase.
  - Ranking batch-sharded: each core ranks only its 4 batches, then ONE
    AllToAll redistributes (sorted key, slot) from batch-sharded to
    rank-sharded layout - exactly what each core needs (its 64 ranks x all
    32 batches).
  - Gather uses dma_gather(transpose=True) on fp16 rows: fuses the memory
    row gather with the pred_in^T transpose (no PE transposes, no repack),
    in 8 chunks that pipeline with the main matmul.
  - W1/W2/Wout preloaded fp16-resident at t=0; dense tail runs right after
    the partial-h AllReduce.

The same program runs on all 8 cores (SPMD); all per-core differences are
carried by per-core input constants (W0 shard, selection/offset constants).
"""

import sys

sys.path.insert(0, "/opt/trn_rl_repo")

import numpy as np

import concourse.bass as bass
import concourse.bacc as bacc
import concourse.mybir as mybir
from concourse import tile
from concourse import bass_utils

F32 = mybir.dt.float32
F16 = mybir.dt.float16
I16 = mybir.dt.int16
U8 = mybir.dt.uint8
ALU = mybir.AluOpType
ACTF = mybir.ActivationFunctionType

B, M, V, H, TD = 32, 512, 256, 1024, 10
NC = 8
RPC = M // NC              # 64 ranks per core
BPC = B // NC              # 4 batches ranked per core
MEMROWS = B * M            # 16384
MEMP = MEMROWS + B         # 16416 gather-source rows (memory rows + x rows)
W0S_ROWS = RPC * V + RPC * TD   # 17024 = 133*128
NKT = W0S_ROWS // 128      # 133 k-tiles (128 mem + 5 bits)
NBT = RPC * TD // 128      # 5 bits k-tiles
NST = 59                   # W0 staging tiles in SBUF
KSPLIT = 67                # main-loop k-tile split for the pipelined AllReduce
NIDX = RPC * B             # 2048 gather items
GCH = 8                    # gather chunks
IPG = NIDX // GCH          # 256 idxs per chunk


def build_program(stage="full"):
    lvl = {"rank": 0, "skb": 1, "gt": 2, "full": 3}[stage]
    nc = bacc.Bacc(
        "TRN2",
        target_bir_lowering=False,
        debug=False,
        enable_asserts=False,
        num_devices=NC,
    )

    def din(name, shape, dtype=F32):
        return nc.dram_tensor(name, list(shape), dtype, kind="ExternalInput").ap()

    memp = din("memp", (MEMP, V), F16)
    timings = din("timings", (B, M))
    msur = din("msur", (B, M))
    w0s = din("W0s", (W0S_ROWS, H), F16)
    w1 = din("W1h", (H, H), F16)
    w2 = din("W2h", (H, H), F16)
    wout = din("Wouth", (H, V), F16)
    b0r = din("b0r", (B, H))
    b1r = din("b1r", (B, H))
    b2r = din("b2r", (B, H))
    boutr = din("boutr", (B, V))
    c_iota = din("c_iota", (B, M))
    c_rrow = din("c_rrow512", (128, M))
    c_eselc = din("c_eselc", (B, 4 * 128))
    c_esel4x = din("c_esel4x", (B, 4))
    c_eye = din("c_eye", (B, B))
    c_slotnum = din("c_slotnum", (128, 4))
    c_selsk = din("c_selsk", (2 * B, B), F16)
    c_selj = din("c_selj64", (2 * B, 2 * 128))
    c_skbsel = din("c_skbsel", (128, 4 * RPC))
    c_amask = din("c_amask", (128, 128))
    c_coff = din("c_coff", (128, 128))

    out = nc.dram_tensor("out", [B, V], F32, kind="ExternalOutput").ap()
    dbg = (nc.dram_tensor("dbg", [128, 512], F32, kind="ExternalOutput").ap()
           if stage != "full" else None)

    with tile.TileContext(nc) as tc:
        with (
            tc.tile_pool(name="const", bufs=1) as constp,
            tc.tile_pool(name="state", bufs=1) as state,
            tc.tile_pool(name="w0t", bufs=NST) as w0p,
            tc.tile_pool(name="gt", bufs=GCH) as gtp,
            tc.tile_pool(name="krep", bufs=2) as krepp,
            tc.tile_pool(name="pt", bufs=2) as ptp,
            tc.tile_pool(name="kslot", bufs=2) as kslotp,
            tc.tile_pool(name="pkrep", bufs=1, space="PSUM") as pkrepp,
            tc.tile_pool(name="psort", bufs=1, space="PSUM") as psortp,
            tc.tile_pool(name="pmisc", bufs=2, space="PSUM") as pmiscp,
            tc.tile_pool(name="ph", bufs=2, space="PSUM") as php,
            tc.tile_pool(name="dram", bufs=1, space="DRAM") as dramp,
        ):
            # ---- W0 stream: issue ALL tile DMAs up-front on the sync HW
            # queue. First NST run at t=0 into the staging pool; the rest
            # stall on slot recycling and stream as the main loop consumes.
            w0_tiles = []
            for kt in range(NKT):
                t = w0p.tile([128, H], F16, tag="w0t")
                nc.sync.dma_start(t[:], w0s[kt * 128:(kt + 1) * 128, :])
                w0_tiles.append(t)

            # ---- constants / state / tail weights on the scalar HW queue
            def load(pool, ap, dtype=None):
                t = pool.tile(list(ap.shape), dtype or ap.dtype,
                              tag=f"ld_{ap.tensor.name}")
                nc.scalar.dma_start(t[:], ap)
                return t

            iota = load(constp, c_iota)
            t_sb = load(state, timings)
            ms_sb = load(state, msur)
            eselc = load(constp, c_eselc)
            esel4x = load(constp, c_esel4x)
            rrow = load(constp, c_rrow)
            slotnum = load(constp, c_slotnum)
            selsk = load(constp, c_selsk)
            selj = load(constp, c_selj)
            skbsel = load(constp, c_skbsel)
            amask = load(constp, c_amask)
            coff = load(constp, c_coff)
            eye = load(constp, c_eye)
            b0s = load(constp, b0r)
            b1s = load(constp, b1r)
            b2s = load(constp, b2r)
            bouts = load(constp, boutr)

            # ---- stage A: keys (all 32 batches; cheap) ------------------
            msur2 = state.tile([B, M], F32, tag="msur2")
            nc.vector.tensor_scalar(msur2[:], ms_sb[:], 0.9, None, ALU.mult)
            minv = state.tile([B, 1], F32, tag="minv")
            nc.vector.tensor_reduce(minv[:], msur2[:], axis=mybir.AxisListType.X,
                                    op=ALU.min)
            mask = state.tile([B, M], U8, tag="mask")
            nc.vector.tensor_scalar(mask[:], msur2[:], minv[:], None, ALU.is_equal)
            cand = state.tile([B, M], F32, tag="cand")
            nc.vector.memset(cand[:], 1.0e9)
            nc.vector.copy_predicated(cand[:], mask[:], iota[:])
            idx0 = state.tile([B, 1], F32, tag="idx0")
            nc.vector.tensor_reduce(idx0[:], cand[:], axis=mybir.AxisListType.X,
                                    op=ALU.min)

            keys = state.tile([B, M], F32, tag="keys")
            nc.vector.tensor_scalar(keys[:], t_sb[:], 512.0, 512.0, ALU.mult,
                                    ALU.add)
            nc.vector.tensor_tensor(keys[:], keys[:], iota[:], ALU.add)
            mask2 = state.tile([B, M], U8, tag="mask2")
            nc.vector.tensor_scalar(mask2[:], iota[:], idx0[:], None, ALU.is_equal)
            nc.vector.copy_predicated(keys[:], mask2[:], iota[:])

            # ---- stage B: rank my 4 batches; extract (sorted key, slot) --
            rank_sb = state.tile([128, 4 * BPC], F32, tag="rank")
            scratch = state.tile([128, M], F32, tag="scratch")
            psort_sb = state.tile([2, BPC * M], F32, tag="psort_sb")
            for bi in range(BPC):
                pk = pkrepp.tile([128, M], F32, tag="pkrep")
                nc.tensor.matmul(pk[:], eselc[:, bi * 128:(bi + 1) * 128],
                                 keys[:], start=True, stop=True)
                krep = krepp.tile([128, M], F32, tag="krep")
                nc.vector.tensor_copy(krep[:], pk[:])
                # keysT for my batch: kslot even cols = key^T, odd = slot ids
                pkt = pmiscp.tile([128, 4], F32, tag="pm")
                for mt in range(4):
                    nc.tensor.matmul(pkt[:, mt:mt + 1],
                                     keys[:, mt * 128:(mt + 1) * 128],
                                     esel4x[:, bi:bi + 1], start=True, stop=True)
                kslot = kslotp.tile([128, 8], F32, tag="kslot")
                ksv = kslot.rearrange("p (c two) -> p c two", two=2)
                nc.vector.tensor_copy(ksv[:, :, 0], pkt[:])
                nc.vector.tensor_copy(ksv[:, :, 1], slotnum[:])
                psort_ps = psortp.tile([2, M], F32, tag="psort")
                for mt in range(4):
                    col = bi * 4 + mt
                    nc.vector.tensor_scalar(
                        scratch[:], krep[:], kslot[:, 2 * mt:2 * mt + 1], None,
                        ALU.is_lt, ALU.add,
                        accum_out=rank_sb[:, col:col + 1])
                    pt = ptp.tile([128, M], F32, tag="pt")
                    nc.vector.tensor_scalar(pt[:], rrow[:],
                                            rank_sb[:, col:col + 1], None,
                                            ALU.is_equal)
                    nc.tensor.matmul(psort_ps[:],
                                     kslot[:, 2 * mt:2 * mt + 2], pt[:],
                                     start=(mt == 0), stop=(mt == 3))
                nc.scalar.activation(
                    psort_sb.rearrange("two (d bi r) -> two d bi r",
                                       d=NC, bi=BPC)[:, :, bi, :],
                    psort_ps.rearrange("two (d r) -> two d r", d=NC),
                    ACTF.Copy)

            if stage == "rank":
                for q in range(4):
                    nc.sync.dma_start(dbg[2 * q:2 * q + 2, 0:512],
                                      psort_sb[:, 512 * q:512 * (q + 1)])
                nc.sync.dma_start(dbg[8:8 + 128 - 120, 0:16], rank_sb[0:8, :])

            # ---- stage C: AllGather (sk, slot), then per-core row select --
            # cc_in flat rows (within a src block): two*32 + d*4 + bi
            cc_in = dramp.tile([2, NC, BPC * RPC], F32, tag="cc_in")
            cc_out = dramp.tile([4, 128, RPC], F32, tag="cc_out")
            nc.scalar.dma_start(
                cc_in[:],
                psort_sb.rearrange("two (d br) -> two d br", d=NC))
            nc.gpsimd.collective_compute(
                "AllGather", ALU.bypass,
                replica_groups=[list(range(NC))],
                ins=[cc_in.opt()],
                outs=[cc_out.opt()],
            )
            # skall[p, g*64+r] = gathered row (g*128+p) = (src,two,d,bi) data
            skall = state.tile([128, 4 * RPC], F32, tag="skall")
            nc.scalar.dma_start(skall.rearrange("p (g r) -> p g r", g=4),
                                cc_out.rearrange("g p r -> p g r"))
            # skb rows: (src, two, bi) -> 8*src + 4*two + bi; cols: my 64 ranks
            pskb = pmiscp.tile([2 * B, RPC], F32, tag="pm")
            for g in range(4):
                nc.tensor.matmul(pskb[:], skbsel[:, g * RPC:(g + 1) * RPC],
                                 skall[:, g * RPC:(g + 1) * RPC],
                                 start=(g == 0), stop=(g == 3))
            skb = state.tile([2 * B, RPC], F32, tag="skb")
            nc.vector.tensor_copy(skb[:], pskb[:])

            # ---- stage D: gather idx + timing bits -----------------------
            # idx[p, 2r+j] = 512*b + slot(b, my rank r), b = p%16 + 16j
            idx_sb = state.tile([128, 128], I16, tag="idx")
            idxv = idx_sb.rearrange("p (r j) -> p r j", j=2)
            itmp = state.tile([128, RPC], F32, tag="itmp")
            for j in range(2):
                pidx = pmiscp.tile([128, RPC], F32, tag="pm")
                nc.tensor.matmul(pidx[:], selj[:, j * 128:(j + 1) * 128],
                                 skb[:], start=True, stop=True)
                nc.vector.tensor_tensor(itmp[:], pidx[:],
                                        amask[:, j * RPC:(j + 1) * RPC], ALU.mult)
                nc.vector.tensor_tensor(idxv[:, :, j], itmp[:],
                                        coff[:, j * RPC:(j + 1) * RPC], ALU.add)

            # Timing-bit extraction: MSB-first binary cascade on skb rows
            # (slot rows produce harmless garbage, never selected).
            # u_all[row, rl*10+d] = bit (9+d) of sk(batch(row), rank 64c+rl)
            u_all = state.tile([2 * B, RPC * TD], F16, tag="u_all")
            u_v = u_all.rearrange("row (rl d) -> row rl d", d=TD)
            rem = state.tile([2 * B, RPC], F32, tag="rem")
            tmpu = state.tile([2 * B, RPC], F32, tag="tmpu")
            nc.vector.tensor_copy(rem[:], skb[:])
            for j in range(18, 8, -1):
                ud = u_v[:, :, j - 9]
                nc.vector.tensor_scalar(ud, rem[:], float(2 ** j), None, ALU.is_ge)
                nc.vector.tensor_scalar(tmpu[:], ud, float(2 ** j), None, ALU.mult)
                nc.vector.tensor_tensor(rem[:], rem[:], tmpu[:], ALU.subtract)

            # bits_sb[p, tb*32+b] = u_all[skrow(b), tb*128+p]
            bits_sb = state.tile([128, NBT * B], F16, tag="bits")
            for tb in range(NBT):
                psr = pmiscp.tile([128, B], F32, tag="pm")
                nc.tensor.matmul(psr[:], u_all[:, tb * 128:(tb + 1) * 128],
                                 selsk[:], start=True, stop=True)
                nc.vector.tensor_copy(bits_sb[:, tb * B:(tb + 1) * B], psr[:])

            if stage == "skb":
                nc.sync.dma_start(dbg[0:64, 0:64], skb[:])
                skf = state.tile([128, 128], F32, tag="skf")
                nc.vector.tensor_copy(skf[:], idx_sb[:])
                nc.sync.dma_start(dbg[0:128, 64:192], skf[:])
                bitf = state.tile([128, NBT * B], F32, tag="bitf")
                nc.vector.tensor_copy(bitf[:], bits_sb[:])
                nc.sync.dma_start(dbg[0:128, 192:352], bitf[:])

            # ---- stage E: fused transpose-gather of memory rows ----------
            gts = []
            for k in range(GCH):
                gt = gtp.tile([128, 2, IPG], F16, tag="gt")
                nc.gpsimd.dma_gather(
                    out_ap=gt[:],
                    in_ap=memp,
                    idxs_ap=idx_sb[:, 16 * k:16 * (k + 1)],
                    num_idxs=IPG,
                    num_idxs_reg=IPG,
                    elem_size=V,
                    transpose=True,
                    single_packet=False,
                )
                gts.append(gt)

            if stage == "gt":
                gtf = state.tile([128, 512], F32, tag="gtf")
                nc.vector.tensor_copy(gtf[:], gts[0].rearrange("p c i -> p (c i)"))
                nc.sync.dma_start(dbg[:, 0:512], gtf[:])

            # ---- stage F: main matmul  partial_h = pred_in_shard @ W0s ---
            # Two psum accumulators split at KSPLIT; the first half's
            # AllReduce is issued mid-stream and hides under the second half.
            ph_1 = php.tile([B, H], F32, tag="ph")
            ph_2 = php.tile([B, H], F32, tag="ph")
            ar_in1 = dramp.tile([B, H], F32, tag="ar_in1")
            ar_out1 = dramp.tile([B, H], F32, tag="ar_out1")
            ar_in2 = dramp.tile([B, H], F32, tag="ar_in2")
            ar_out2 = dramp.tile([B, H], F32, tag="ar_out2")
            part1 = state.tile([B, H], F32, tag="part1")
            part2 = state.tile([B, H], F32, tag="part2")
            for kt in range(NKT):
                if kt < 2 * RPC:
                    r, h2 = kt // 2, kt % 2
                    gt = gts[r // 8]
                    rr = r % 8
                    lhsT = gt[:, h2, rr * B:(rr + 1) * B]
                else:
                    tb = kt - 2 * RPC
                    lhsT = bits_sb[:, tb * B:(tb + 1) * B]
                ph_t = ph_1 if kt < KSPLIT else ph_2
                first = kt == 0 or kt == KSPLIT
                last = kt == KSPLIT - 1 or kt == NKT - 1
                w0t = w0_tiles[kt]
                nc.tensor.matmul(ph_t[:, 0:512], lhsT, w0t[:, 0:512],
                                 start=first, stop=last)
                nc.tensor.matmul(ph_t[:, 512:1024], lhsT, w0t[:, 512:1024],
                                 start=first, stop=last)
                if kt == KSPLIT - 1:
                    nc.vector.tensor_copy(part1[:], ph_1[:])
                    nc.scalar.dma_start(ar_in1[:], part1[:])
                    nc.gpsimd.collective_compute(
                        "AllReduce", ALU.add,
                        replica_groups=[list(range(NC))],
                        ins=[ar_in1.opt()], outs=[ar_out1.opt()])

            # ---- stage G: AllReduce second half, combine -----------------
            nc.vector.tensor_copy(part2[:], ph_2[:])
            nc.scalar.dma_start(ar_in2[:], part2[:])
            nc.gpsimd.collective_compute(
                "AllReduce", ALU.add,
                replica_groups=[list(range(NC))],
                ins=[ar_in2.opt()], outs=[ar_out2.opt()])
            # tail weights ride the tail of the W0 staging pool rotation
            w1t, w2t, wot = [], [], []
            for kt in range(8):
                t = w0p.tile([128, H], F16, tag="w0t")
                nc.sync.dma_start(t[:], w1[kt * 128:(kt + 1) * 128, :])
                w1t.append(t)
            for kt in range(8):
                t = w0p.tile([128, H], F16, tag="w0t")
                nc.sync.dma_start(t[:], w2[kt * 128:(kt + 1) * 128, :])
                w2t.append(t)
            for half in range(2):
                t = w0p.tile([128, H], F16, tag="w0t")
                nc.sync.dma_start(
                    t.rearrange("p (k f) -> p k f", k=4),
                    wout.rearrange("(k p) f -> p k f", k=8)
                    [:, 4 * half:4 * (half + 1), :])
                wot.append(t)
            h_sb = state.tile([B, H], F32, tag="h_sb")
            hb2 = state.tile([B, H], F32, tag="hb2")
            nc.scalar.dma_start(h_sb[:], ar_out1[:])
            nc.scalar.dma_start(hb2[:], ar_out2[:])
            nc.vector.tensor_tensor(h_sb[:], h_sb[:], hb2[:], ALU.add)
            nc.vector.tensor_tensor(h_sb[:], h_sb[:], b0s[:], ALU.add)
            nc.vector.tensor_scalar(h_sb[:], h_sb[:], 0.0, None, ALU.max)

            # ---- stage H: dense tail (replicated on every core) ----------
            def dense(h_in, rhs_fn, bias_sb, n_out, relu, tag):
                hT = state.tile([128, 8 * B], F16, tag=f"hT_{tag}")
                for kt in range(8):
                    ptt = pmiscp.tile([128, B], F32, tag="pm")
                    nc.tensor.transpose(ptt[:], h_in[:, kt * 128:(kt + 1) * 128],
                                        eye[:])
                    nc.vector.tensor_copy(hT[:, kt * B:(kt + 1) * B], ptt[:])
                pho = php.tile([B, n_out], F32, tag="ph")
                for kt in range(8):
                    for j0 in range(0, n_out, 512):
                        jn = min(512, n_out - j0)
                        nc.tensor.matmul(
                            pho[:, j0:j0 + jn], hT[:, kt * B:(kt + 1) * B],
                            rhs_fn(kt, j0, jn),
                            start=(kt == 0), stop=(kt == 7))
                h_next = state.tile([B, n_out], F32, tag=f"h_{tag}")
                nc.vector.tensor_tensor(h_next[:], pho[:], bias_sb[:], ALU.add)
                if relu:
                    nc.vector.tensor_scalar(h_next[:], h_next[:], 0.0, None,
                                            ALU.max)
                return h_next

            h1 = dense(h_sb, lambda kt, j0, jn: w1t[kt][:, j0:j0 + jn],
                       b1s, H, True, "l1")
            h2 = dense(h1, lambda kt, j0, jn: w2t[kt][:, j0:j0 + jn],
                       b2s, H, True, "l2")
            logits = dense(
                h2, lambda kt, j0, jn: wot[kt // 4][:, (kt % 4) * V:(kt % 4 + 1) * V],
                bouts, V, False, "lo")
            nc.scalar.dma_start(out, logits[:])

    nc.compile()
    return nc


def make_in_maps(inputs):
    x = np.asarray(inputs["x"], np.float32)
    memory = np.asarray(inputs["memory"], np.float32)
    timings = np.asarray(inputs["memory_timings"], np.float32)
    msur = np.asarray(inputs["memory_surprise"], np.float32)
    W0 = np.asarray(inputs["W0"], np.float32)
    W1 = np.asarray(inputs["W1"], np.float32)
    W2 = np.asarray(inputs["W2"], np.float32)
    Wout = np.asarray(inputs["Wout"], np.float32)
    b0 = np.asarray(inputs["b0"], np.float32)
    b1 = np.asarray(inputs["b1"], np.float32)
    b2 = np.asarray(inputs["b2"], np.float32)
    bout = np.asarray(inputs["bout"], np.float32)

    memp = np.concatenate([memory.reshape(MEMROWS, V), x], 0).astype(np.float16)

    p = np.arange(128)

    # shared constants
    iota = np.broadcast_to(np.arange(M, dtype=np.float32), (B, M)).copy()
    rrow = np.broadcast_to(np.arange(M, dtype=np.float32), (128, M)).copy()
    eye = np.eye(B, dtype=np.float32)
    slotnum = np.empty((128, 4), np.float32)
    for mt in range(4):
        slotnum[:, mt] = p + mt * 128
    # skb rows: (src, two, bi) -> 8*src + 4*two + bi
    # selsk[row, b] = [row == sk-row of batch b]  (sk row = 8*(b//4)+(b%4))
    selsk = np.zeros((2 * B, B), np.float16)
    for b in range(B):
        selsk[8 * (b // 4) + (b % 4), b] = 1.0
    # selj[row, j*128+p] = [row == slot-row of batch p%16+16j]
    selj = np.zeros((2 * B, 2 * 128), np.float32)
    for j in range(2):
        for pp in range(128):
            b = pp % 16 + 16 * j
            selj[8 * (b // 4) + 4 + (b % 4), j * 128 + pp] = 1.0
    shared = {
        "memp": memp,
        "timings": timings,
        "msur": msur,
        "W1h": W1.astype(np.float16),
        "W2h": W2.astype(np.float16),
        "Wouth": Wout.astype(np.float16),
        "b0r": np.broadcast_to(b0, (B, H)).copy(),
        "b1r": np.broadcast_to(b1, (B, H)).copy(),
        "b2r": np.broadcast_to(b2, (B, H)).copy(),
        "boutr": np.broadcast_to(bout, (B, V)).copy(),
        "c_iota": iota, "c_rrow512": rrow, "c_eye": eye,
        "c_slotnum": slotnum, "c_selsk": selsk, "c_selj64": selj,
    }

    in_maps = []
    for core in range(NC):
        w0shard = np.concatenate(
            [W0[core * RPC * V:(core + 1) * RPC * V],
             W0[M * V + core * RPC * TD: M * V + (core + 1) * RPC * TD]],
            0).astype(np.float16)
        eselc = np.zeros((B, 4 * 128), np.float32)
        esel4x = np.zeros((B, 4), np.float32)
        for bi in range(BPC):
            eselc[4 * core + bi, bi * 128:(bi + 1) * 128] = 1.0
            esel4x[4 * core + bi, bi] = 1.0
        # skbsel[p, g*64+orow] = [g*128+p == src*64 + two*32 + core*4 + bi]
        # where orow = 8*src + 4*two + bi
        skbsel = np.zeros((128, 4 * RPC), np.float32)
        for orow in range(64):
            src, two, bi = orow // 8, (orow % 8) // 4, orow % 4
            row = src * 64 + two * 32 + core * 4 + bi
            skbsel[row % 128, (row // 128) * 64 + orow] = 1.0
        amask = np.ones((128, 128), np.float32)
        coffm = np.empty((128, 128), np.float32)
        for j in range(2):
            bcol = (p % 16 + 16 * j).astype(np.float32)
            coffm[:, j * RPC:(j + 1) * RPC] = (512.0 * bcol)[:, None]
            if core == 0:
                amask[:, j * RPC] = 0.0
                coffm[:, j * RPC] = MEMROWS + bcol
        m = dict(shared)
        m["W0s"] = np.ascontiguousarray(w0shard)
        m["c_eselc"] = eselc
        m["c_esel4x"] = esel4x
        m["c_skbsel"] = skbsel
        m["c_amask"] = amask
        m["c_coff"] = coffm
        in_maps.append(m)
    return in_maps


_NC_CACHE = None


def kernel(**inputs) -> np.ndarray:
    global _NC_CACHE
    if _NC_CACHE is None:
        _NC_CACHE = build_program()
    nc = _NC_CACHE
    in_maps = make_in_maps(inputs)
    res = bass_utils.run_bass_kernel_spmd(nc, in_maps, core_ids=list(range(NC)))
    return np.asarray(res.results[0]["out"], np.float32)


if __name__ == "__main__":
    np.random.seed(0)
    build_program(sys.argv[1] if len(sys.argv) > 1 else "full")
    print("build OK")
